# revision 7
# baseline (speedup 1.0000x reference)
"""DeepSeek-style MoE layer on 8 Trainium2 NeuronCores (expert-parallel).

Sharding: core e owns routed expert e (full SwiGLU weights for that expert)
plus a 512-token slice of the shared expert. Every core computes the fp32
router for all 4096 tokens on-device, top-2 via the DVE sort8 unit, compacts
its own expert's token list with gpsimd sparse_gather, gathers those token
rows with dma_gather (transposing gather -> contraction-major layout), runs
the expert SwiGLU in bf16, and scales rows by the combine weights. The host
only lays out inputs and scatter-adds the per-core results back together.
"""

import os
import numpy as np
import ml_dtypes

import concourse.bass as bass
import concourse.bacc as bacc
import concourse.mybir as mybir
import concourse.tile as tile
from concourse.bass_utils import run_bass_kernel_spmd
from contextlib import ExitStack

BF16 = ml_dtypes.bfloat16
F32 = mybir.dt.float32
BF = mybir.dt.bfloat16
U32 = mybir.dt.uint32
I16 = mybir.dt.int16

P = 128
H = 1024
I = 4096
E = 8
T = 4096
KH = H // P           # 8 contraction chunks over H
NI = I // P           # 32 I-tiles
TSH = T // E          # 512 tokens/core for the shared expert
CPAD = 1536           # compact-token capacity per expert (max real load ~1071)
NCH = CPAD // 512     # gather / rhs chunks
CW = CPAD // 16       # 16-wrapped free dim of compact buffers (96)

AF = mybir.ActivationFunctionType
ALU = mybir.AluOpType

_NC_CACHE = {}


def _build_nc():
    nc = bacc.Bacc(None, target_bir_lowering=False, debug=False)

    # inputs (per-core maps supply the data; shapes identical across cores)
    xr_h = nc.dram_tensor("xr", [P, KH, T], F32, kind="ExternalInput")
    xb_h = nc.dram_tensor("xb", [T, H], BF, kind="ExternalInput")
    xs_h = nc.dram_tensor("xs", [P, KH, TSH], BF, kind="ExternalInput")
    rw_h = nc.dram_tensor("rwt", [P, KH, E], F32, kind="ExternalInput")
    gw_h = nc.dram_tensor("gw", [NI, P, H], BF, kind="ExternalInput")
    uw_h = nc.dram_tensor("uw", [NI, P, H], BF, kind="ExternalInput")
    dw_h = nc.dram_tensor("dw", [NI, P, H], BF, kind="ExternalInput")
    sgw_h = nc.dram_tensor("sgw", [NI, P, H], BF, kind="ExternalInput")
    suw_h = nc.dram_tensor("suw", [NI, P, H], BF, kind="ExternalInput")
    sdw_h = nc.dram_tensor("sdw", [NI, P, H], BF, kind="ExternalInput")
    eid_h = nc.dram_tensor("eid", [P, 1], F32, kind="ExternalInput")
    tok_h = nc.dram_tensor("tok1", [P, 32], F32, kind="ExternalInput")
    one_h = nc.dram_tensor("ones", [P, 1], F32, kind="ExternalInput")

    # outputs
    y_h = nc.dram_tensor("y", [CPAD, H], F32, kind="ExternalOutput")
    cidx_h = nc.dram_tensor("cidx", [16, CW], F32, kind="ExternalOutput")
    cnt_h = nc.dram_tensor("cnt", [1, 1], U32, kind="ExternalOutput")
    ysh_h = nc.dram_tensor("ysh", [TSH, H], F32, kind="ExternalOutput")
    zs_h = nc.dram_tensor("zs", [1, 1], F32, kind="ExternalOutput")

    with tile.TileContext(nc) as tc, ExitStack() as ctx:
        const = ctx.enter_context(tc.tile_pool(name="const", bufs=1))
        rt = ctx.enter_context(tc.tile_pool(name="rt", bufs=1))
        xrp = ctx.enter_context(tc.tile_pool(name="xrp", bufs=3))
        wp = ctx.enter_context(tc.tile_pool(name="wp", bufs=6))
        actp = ctx.enter_context(tc.tile_pool(name="actp", bufs=3))
        outp = ctx.enter_context(tc.tile_pool(name="outp", bufs=3))
        xgp = ctx.enter_context(tc.tile_pool(name="xgp", bufs=1))

        # ---- constants -------------------------------------------------
        rw_sb = const.tile([P, KH, E], F32)
        nc.sync.dma_start(rw_sb[:], rw_h[:, :, :])
        eid_sb = const.tile([P, 1], F32)
        nc.sync.dma_start(eid_sb[:], eid_h[:, :])
        tok_sb = const.tile([P, 32], F32)
        nc.sync.dma_start(tok_sb[:], tok_h[:, :])
        one_sb = const.tile([P, 1], F32)
        nc.sync.dma_start(one_sb[:], one_h[:, :])
        xs_sb = const.tile([P, KH, TSH], BF)
        nc.sync.dma_start(xs_sb[:], xs_h[:, :, :])

        # ---- router: fp32 logits for all tokens ------------------------
        lg = rt.tile([P, 32 * E], F32)
        with tc.tile_pool(name="psR", bufs=4, space="PSUM") as psR:
            for j in range(32):
                xc = xrp.tile([P, KH, P], F32, tag="xc")
                nc.sync.dma_start(xc[:], xr_h[:, :, j * P:(j + 1) * P])
                ps = psR.tile([P, E], F32, tag="r")
                for k in range(KH):
                    nc.tensor.matmul(ps[:], xc[:, k, :], rw_sb[:, k, :],
                                     start=(k == 0), stop=(k == KH - 1))
                nc.scalar.copy(lg[:, j * E:(j + 1) * E], ps[:])

            # router z-loss: sum of squared logits
            sq = rt.tile([P, 32 * E], F32)
            zac = rt.tile([P, 1], F32)
            nc.scalar.activation(sq[:], lg[:], AF.Square, accum_out=zac[:])
            zp = psR.tile([1, 1], F32, tag="z")
            nc.tensor.matmul(zp[:], zac[:], one_sb[:], start=True, stop=True)
            zsb = rt.tile([1, 1], F32)
            nc.scalar.copy(zsb[:], zp[:])
            nc.sync.dma_start(zs_h[:, :], zsb[:])

        # ---- top-2 + combine weights -----------------------------------
        mx = rt.tile([P, 32 * E], F32)
        ix = rt.tile([P, 32 * E], U32)
        for j in range(32):
            nc.vector.max_with_indices(mx[:, j * E:(j + 1) * E],
                                       ix[:, j * E:(j + 1) * E],
                                       lg[:, j * E:(j + 1) * E])
        mx3 = mx[:].rearrange("p (j e) -> p j e", e=E)
        ix3 = ix[:].rearrange("p (j e) -> p j e", e=E)
        i1f = rt.tile([P, 32], F32)
        nc.vector.tensor_copy(i1f[:], ix3[:, :, 0])
        i2f = rt.tile([P, 32], F32)
        nc.vector.tensor_copy(i2f[:], ix3[:, :, 1])
        d12 = rt.tile([P, 32], F32)
        nc.vector.tensor_tensor(d12[:], mx3[:, :, 0], mx3[:, :, 1], ALU.subtract)
        w1 = rt.tile([P, 32], F32)
        nc.scalar.activation(w1[:], d12[:], AF.Sigmoid)
        w2 = rt.tile([P, 32], F32)
        nc.vector.tensor_scalar(w2[:], w1[:], -1.0, 1.0, ALU.mult, ALU.add)
        f1 = rt.tile([P, 32], F32)
        nc.vector.tensor_scalar(f1[:], i1f[:], eid_sb[:], None, ALU.is_equal)
        f2 = rt.tile([P, 32], F32)
        nc.vector.tensor_scalar(f2[:], i2f[:], eid_sb[:], None, ALU.is_equal)
        fany = rt.tile([P, 32], F32)
        nc.vector.tensor_tensor(fany[:], f1[:], f2[:], ALU.add)
        wa = rt.tile([P, 32], F32)
        nc.vector.tensor_tensor(wa[:], f1[:], w1[:], ALU.mult)
        wb = rt.tile([P, 32], F32)
        nc.vector.tensor_tensor(wb[:], f2[:], w2[:], ALU.mult)
        ws = rt.tile([P, 32], F32)
        nc.vector.tensor_tensor(ws[:], wa[:], wb[:], ALU.add)
        fm1 = rt.tile([P, 32], F32)
        nc.vector.tensor_scalar(fm1[:], fany[:], -1.0, None, ALU.add)
        wmask = rt.tile([P, 32], F32)
        nc.vector.tensor_tensor(wmask[:], ws[:], fm1[:], ALU.add)
        idm = rt.tile([P, 32], F32)
        nc.vector.tensor_tensor(idm[:], fany[:], tok_sb[:], ALU.mult)
        imask = rt.tile([P, 32], F32)
        nc.vector.tensor_scalar(imask[:], idm[:], -1.0, None, ALU.add)

        # ---- 16-wrap repack: token t -> (t%16, t//16) ------------------
        ids16 = rt.tile([16, 256], F32)
        w16 = rt.tile([16, 256], F32)
        ids163 = ids16[:].rearrange("p (c a) -> p c a", a=8)
        w163 = w16[:].rearrange("p (c a) -> p c a", a=8)
        for a in range(8):
            nc.sync.dma_start(ids163[:, :, a], imask[16 * a:16 * (a + 1), :])
            nc.sync.dma_start(w163[:, :, a], wmask[16 * a:16 * (a + 1), :])

        # ---- compaction ------------------------------------------------
        cid16 = rt.tile([16, CW], F32)
        cnt_sb = rt.tile([1, 1], U32)
        nc.gpsimd.sparse_gather(cid16[:], ids16[:], num_found=cnt_sb[:])
        cw16 = rt.tile([16, CW], F32)
        cnt2_sb = rt.tile([1, 1], U32)
        nc.gpsimd.sparse_gather(cw16[:], w16[:], num_found=cnt2_sb[:])
        nc.sync.dma_start(cidx_h[:, :], cid16[:])
        nc.sync.dma_start(cnt_h[:, :], cnt_sb[:])

        # sanitize (-1 pad -> 0) and replicate indices for dma_gather
        cidm = rt.tile([16, CW], F32)
        nc.vector.tensor_scalar(cidm[:], cid16[:], 0.0, None, ALU.max)
        cid16i = rt.tile([16, CW], I16)
        nc.vector.tensor_copy(cid16i[:], cidm[:])
        cidi = rt.tile([P, CW], I16)
        for g in range(8):
            nc.sync.dma_start(cidi[16 * g:16 * (g + 1), :], cid16i[:])
        # combine weights -> slot-major [128, CPAD/128] layout
        cwP = rt.tile([P, CPAD // P], F32)
        cw3 = cw16[:].rearrange("p (k c) -> p k c", c=8)
        for c in range(8):
            nc.sync.dma_start(cwP[16 * c:16 * (c + 1), :], cw3[:, :, c])

        # ---- gather selected token rows (transposing gather) -----------
        xg = []
        for c in range(NCH):
            t = xgp.tile([P, KH, 512], BF, tag=f"xg{c}", name=f"xg{c}")
            nc.gpsimd.dma_gather(
                t[:], xb_h[:, :], cidi[:, 32 * c:32 * (c + 1)],
                num_idxs=512, num_idxs_reg=512, elem_size=H, transpose=True)
            xg.append(t)

        # ---- shared expert on this core's 512-token slice --------------
        with tc.tile_pool(name="ishp", bufs=1) as ishp:
            ish = ishp.tile([P, NI, TSH], BF)
            with tc.tile_pool(name="psG", bufs=2, space="PSUM") as psG:
                for i in range(NI):
                    gwc = wp.tile([P, H], BF, tag="w")
                    nc.sync.dma_start(gwc[:], sgw_h[i, :, :])
                    uwc = wp.tile([P, H], BF, tag="w")
                    nc.sync.dma_start(uwc[:], suw_h[i, :, :])
                    gps = psG.tile([P, TSH], F32, tag="g")
                    ups = psG.tile([P, TSH], F32, tag="u")
                    for k in range(KH):
                        nc.tensor.matmul(gps[:], gwc[:, k * P:(k + 1) * P],
                                         xs_sb[:, k, :],
                                         start=(k == 0), stop=(k == KH - 1))
                    for k in range(KH):
                        nc.tensor.matmul(ups[:], uwc[:, k * P:(k + 1) * P],
                                         xs_sb[:, k, :],
                                         start=(k == 0), stop=(k == KH - 1))
                    sg = actp.tile([P, TSH], F32, tag="sg")
                    nc.scalar.activation(sg[:], gps[:], AF.Sigmoid)
                    t1 = actp.tile([P, TSH], F32, tag="t1")
                    nc.vector.tensor_tensor(t1[:], sg[:], gps[:], ALU.mult)
                    nc.vector.tensor_tensor(ish[:, i, :], t1[:], ups[:], ALU.mult)
            with tc.tile_pool(name="psDs", bufs=1, space="PSUM") as psD:
                dps = [psD.tile([P, 512], F32, tag=f"d{q}", name=f"dps{q}") for q in range(8)]
                for i in range(NI):
                    dwc = wp.tile([P, H], BF, tag="w")
                    nc.sync.dma_start(dwc[:], sdw_h[i, :, :])
                    for cs in range(4):
                        lhs = ish[:, i, cs * P:(cs + 1) * P]
                        for hh in range(2):
                            nc.tensor.matmul(dps[2 * cs + hh][:], lhs,
                                             dwc[:, hh * 512:(hh + 1) * 512],
                                             start=(i == 0), stop=(i == NI - 1))
                for cs in range(4):
                    ysb = outp.tile([P, H], F32, tag="ysh")
                    for hh in range(2):
                        nc.scalar.copy(ysb[:, hh * 512:(hh + 1) * 512],
                                       dps[2 * cs + hh][:])
                    nc.sync.dma_start(ysh_h[cs * P:(cs + 1) * P, :], ysb[:])

        # ---- routed expert: gate/up on compact tokens ------------------
        with tc.tile_pool(name="innp", bufs=1) as innp:
            inner = innp.tile([P, NI * CPAD], BF)
            innv = inner[:].rearrange("p (i c) -> p i c", c=CPAD)
            with tc.tile_pool(name="psGU", bufs=1, space="PSUM") as psGU:
                for i in range(NI):
                    gwc = wp.tile([P, H], BF, tag="w")
                    nc.sync.dma_start(gwc[:], gw_h[i, :, :])
                    uwc = wp.tile([P, H], BF, tag="w")
                    nc.sync.dma_start(uwc[:], uw_h[i, :, :])
                    gp = [psGU.tile([P, 512], F32, tag=f"g{c}", name=f"gp{c}") for c in range(NCH)]
                    up = [psGU.tile([P, 512], F32, tag=f"u{c}", name=f"up{c}") for c in range(NCH)]
                    for k in range(KH):
                        for c in range(NCH):
                            nc.tensor.matmul(gp[c][:], gwc[:, k * P:(k + 1) * P],
                                             xg[c][:, k, :],
                                             start=(k == 0), stop=(k == KH - 1))
                    for k in range(KH):
                        for c in range(NCH):
                            nc.tensor.matmul(up[c][:], uwc[:, k * P:(k + 1) * P],
                                             xg[c][:, k, :],
                                             start=(k == 0), stop=(k == KH - 1))
                    for c in range(NCH):
                        sg = actp.tile([P, 512], F32, tag="sg")
                        nc.scalar.activation(sg[:], gp[c][:], AF.Sigmoid)
                        t1 = actp.tile([P, 512], F32, tag="t1")
                        nc.vector.tensor_tensor(t1[:], sg[:], gp[c][:], ALU.mult)
                        nc.vector.tensor_tensor(innv[:, i, c * 512:(c + 1) * 512],
                                                t1[:], up[c][:], ALU.mult)

            # ---- routed down + combine-weight scaling ------------------
            with tc.tile_pool(name="psDr", bufs=1, space="PSUM") as psD2:
                for grp in range(NCH):
                    dps = [psD2.tile([P, 512], F32, tag=f"d{q}", name=f"dpr{q}") for q in range(8)]
                    for i in range(NI):
                        dwc = wp.tile([P, H], BF, tag="w")
                        nc.sync.dma_start(dwc[:], dw_h[i, :, :])
                        for lcs in range(4):
                            cs = 4 * grp + lcs
                            lhs = innv[:, i, cs * P:(cs + 1) * P]
                            for hh in range(2):
                                nc.tensor.matmul(dps[2 * lcs + hh][:], lhs,
                                                 dwc[:, hh * 512:(hh + 1) * 512],
                                                 start=(i == 0), stop=(i == NI - 1))
                    for lcs in range(4):
                        cs = 4 * grp + lcs
                        ysb = outp.tile([P, H], F32, tag="y")
                        for hh in range(2):
                            nc.vector.tensor_scalar(
                                ysb[:, hh * 512:(hh + 1) * 512],
                                dps[2 * lcs + hh][:], cwP[:, cs:cs + 1], None,
                                ALU.mult)
                        nc.sync.dma_start(y_h[cs * P:(cs + 1) * P, :], ysb[:])

    nc.compile()
    return nc


def get_nc():
    if "nc" not in _NC_CACHE:
        _NC_CACHE["nc"] = _build_nc()
    return _NC_CACHE["nc"]


def _prep_gu(w):
    # [I, H] -> [NI, P, H]; [it, p, k*128+i] = w[128*it+i, 128*k+p]
    return np.ascontiguousarray(
        w.reshape(NI, P, KH, P).transpose(0, 3, 2, 1).reshape(NI, P, H)
    ).astype(BF16)


def _prep_d(w):
    # [H, I] -> [NI, P, H]; [ic, p, h] = w[h, 128*ic+p]
    return np.ascontiguousarray(w.T.reshape(NI, P, H)).astype(BF16)


def prepare_in_maps(hidden_states, router_w, gate_w, up_w, down_w,
                    sh_gate_w, sh_up_w, sh_down_w):
    x = np.asarray(hidden_states, np.float32).reshape(T, H)
    xT = np.ascontiguousarray(x.T)
    xr = np.ascontiguousarray(xT.reshape(KH, P, T).transpose(1, 0, 2))
    xb = x.astype(BF16)
    rw = np.ascontiguousarray(
        np.asarray(router_w, np.float32).T.reshape(KH, P, E).transpose(1, 0, 2))
    sgw = _prep_gu(np.asarray(sh_gate_w, np.float32))
    suw = _prep_gu(np.asarray(sh_up_w, np.float32))
    sdw = _prep_d(np.asarray(sh_down_w, np.float32))
    tok1 = (np.arange(32, dtype=np.float32)[None, :] * P
            + np.arange(P, dtype=np.float32)[:, None] + 1.0)
    ones = np.ones((P, 1), np.float32)
    in_maps = []
    for e in range(E):
        xs = np.ascontiguousarray(
            xT[:, e * TSH:(e + 1) * TSH].reshape(KH, P, TSH).transpose(1, 0, 2)
        ).astype(BF16)
        in_maps.append(dict(
            xr=xr, xb=xb, xs=xs, rwt=rw,
            gw=_prep_gu(np.asarray(gate_w[e], np.float32)),
            uw=_prep_gu(np.asarray(up_w[e], np.float32)),
            dw=_prep_d(np.asarray(down_w[e], np.float32)),
            sgw=sgw, suw=suw, sdw=sdw,
            eid=np.full((P, 1), float(e), np.float32),
            tok1=tok1, ones=ones,
        ))
    return in_maps


def combine_outputs(outs):
    final = np.zeros((T, H), np.float32)
    counts = np.zeros(E, np.float32)
    for e in range(E):
        o = outs[e]
        final[e * TSH:(e + 1) * TSH] = np.asarray(o["ysh"], np.float32)
    for e in range(E):
        o = outs[e]
        cnt = int(np.asarray(o["cnt"]).ravel()[0])
        counts[e] = cnt
        ids = np.asarray(o["cidx"]).reshape(16, CW).T.ravel()[:cnt].astype(np.int64)
        final[ids] += np.asarray(o["y"], np.float32)[:cnt]
    loads = np.concatenate([counts, [float(T)]]).astype(np.float32)
    loads_n = loads / loads.sum()
    lb = np.mean((loads_n - 1.0 / (E + 1)) ** 2)
    z = float(np.asarray(outs[0]["zs"]).ravel()[0]) / float(T)
    aux = np.float32(0.01 * lb + 0.01 * z)
    return final.reshape(2, 2048, H), aux


def kernel(hidden_states, router_w, gate_w, up_w, down_w,
           sh_gate_w, sh_up_w, sh_down_w):
    in_maps = prepare_in_maps(hidden_states, router_w, gate_w, up_w, down_w,
                              sh_gate_w, sh_up_w, sh_down_w)
    nc = get_nc()
    res = run_bass_kernel_spmd(nc, in_maps, list(range(E)),
                               trace=bool(os.environ.get("MOE_TRACE")))
    if os.environ.get("MOE_TRACE"):
        _NC_CACHE["last_results"] = res
    final, aux = combine_outputs(res.results)
    return final, aux


# revision 8
# speedup vs baseline: 1.1063x; 1.1063x over previous
"""DeepSeek-style MoE layer on 8 Trainium2 NeuronCores (expert-parallel).

Sharding: core e owns routed expert e (full SwiGLU weights for that expert)
plus a 512-token slice of the shared expert. Every core computes the fp32
router for all 4096 tokens on-device, top-2 via the DVE sort8 unit, compacts
its own expert's token list with gpsimd sparse_gather, gathers those token
rows with dma_gather (transposing gather -> contraction-major layout), runs
the expert SwiGLU in bf16, and scales rows by the combine weights. The host
only lays out inputs and scatter-adds the per-core results back together.
"""

import os
import numpy as np
import ml_dtypes

import concourse.bass as bass
import concourse.bacc as bacc
import concourse.mybir as mybir
import concourse.tile as tile
from concourse.bass_utils import run_bass_kernel_spmd
from contextlib import ExitStack

BF16 = ml_dtypes.bfloat16
F32 = mybir.dt.float32
BF = mybir.dt.bfloat16
U32 = mybir.dt.uint32
I16 = mybir.dt.int16

P = 128
H = 1024
I = 4096
E = 8
T = 4096
KH = H // P           # 8 contraction chunks over H
NI = I // P           # 32 I-tiles
TSH = T // E          # 512 tokens/core for the shared expert
CPAD = 1280           # compact-token capacity per expert (max real load ~1071)
CHUNKS = [(0, 512), (512, 512), (1024, 256)]   # (offset, len) rhs chunks
NCH = len(CHUNKS)
NCS = CPAD // 128     # 128-token sub-chunks for the down matmul (10)
CSGRP = [(0, 4), (4, 4), (8, 2)]               # (first cs, n cs) down groups
CW = CPAD // 16       # 16-wrapped free dim of compact buffers (80)

AF = mybir.ActivationFunctionType
ALU = mybir.AluOpType

_NC_CACHE = {}


def _build_nc():
    nc = bacc.Bacc(None, target_bir_lowering=False, debug=False)

    # inputs (per-core maps supply the data; shapes identical across cores)
    xr_h = nc.dram_tensor("xr", [P, KH, T], F32, kind="ExternalInput")
    xb_h = nc.dram_tensor("xb", [T, H], BF, kind="ExternalInput")
    xs_h = nc.dram_tensor("xs", [P, KH, TSH], BF, kind="ExternalInput")
    rw_h = nc.dram_tensor("rwt", [P, KH, E], F32, kind="ExternalInput")
    gw_h = nc.dram_tensor("gw", [NI, P, H], BF, kind="ExternalInput")
    uw_h = nc.dram_tensor("uw", [NI, P, H], BF, kind="ExternalInput")
    dw_h = nc.dram_tensor("dw", [NI, P, H], BF, kind="ExternalInput")
    sgw_h = nc.dram_tensor("sgw", [NI, P, H], BF, kind="ExternalInput")
    suw_h = nc.dram_tensor("suw", [NI, P, H], BF, kind="ExternalInput")
    sdw_h = nc.dram_tensor("sdw", [NI, P, H], BF, kind="ExternalInput")
    eid_h = nc.dram_tensor("eid", [P, 1], F32, kind="ExternalInput")
    tok_h = nc.dram_tensor("tok1", [P, 32], F32, kind="ExternalInput")
    one_h = nc.dram_tensor("ones", [P, 1], F32, kind="ExternalInput")

    # outputs
    y_h = nc.dram_tensor("y", [CPAD, H], F32, kind="ExternalOutput")
    cidx_h = nc.dram_tensor("cidx", [16, CW], F32, kind="ExternalOutput")
    cnt_h = nc.dram_tensor("cnt", [1, 1], U32, kind="ExternalOutput")
    ysh_h = nc.dram_tensor("ysh", [TSH, H], F32, kind="ExternalOutput")
    zs_h = nc.dram_tensor("zs", [1, 1], F32, kind="ExternalOutput")

    with tile.TileContext(nc) as tc, ExitStack() as ctx:
        const = ctx.enter_context(tc.tile_pool(name="const", bufs=1))
        rt = ctx.enter_context(tc.tile_pool(name="rt", bufs=1))
        xrp = ctx.enter_context(tc.tile_pool(name="xrp", bufs=3))
        wp = ctx.enter_context(tc.tile_pool(name="wp", bufs=6))
        actp = ctx.enter_context(tc.tile_pool(name="actp", bufs=3))
        outp = ctx.enter_context(tc.tile_pool(name="outp", bufs=3))
        xgp = ctx.enter_context(tc.tile_pool(name="xgp", bufs=1))

        # ---- constants -------------------------------------------------
        rw_sb = const.tile([P, KH, E], F32)
        nc.sync.dma_start(rw_sb[:], rw_h[:, :, :])
        eid_sb = const.tile([P, 1], F32)
        nc.sync.dma_start(eid_sb[:], eid_h[:, :])
        tok_sb = const.tile([P, 32], F32)
        nc.sync.dma_start(tok_sb[:], tok_h[:, :])
        one_sb = const.tile([P, 1], F32)
        nc.sync.dma_start(one_sb[:], one_h[:, :])
        xs_sb = const.tile([P, KH, TSH], BF)
        nc.sync.dma_start(xs_sb[:], xs_h[:, :, :])

        # ---- router: fp32 logits for all tokens ------------------------
        lg = rt.tile([P, 32 * E], F32)
        with tc.tile_pool(name="psR", bufs=4, space="PSUM") as psR:
            for j in range(32):
                xc = xrp.tile([P, KH, P], F32, tag="xc")
                nc.sync.dma_start(xc[:], xr_h[:, :, j * P:(j + 1) * P])
                ps = psR.tile([P, E], F32, tag="r")
                for k in range(KH):
                    nc.tensor.matmul(ps[:], xc[:, k, :], rw_sb[:, k, :],
                                     start=(k == 0), stop=(k == KH - 1))
                nc.scalar.copy(lg[:, j * E:(j + 1) * E], ps[:])

            # router z-loss: sum of squared logits
            sq = rt.tile([P, 32 * E], F32)
            zac = rt.tile([P, 1], F32)
            nc.scalar.activation(sq[:], lg[:], AF.Square, accum_out=zac[:])
            zp = psR.tile([1, 1], F32, tag="z")
            nc.tensor.matmul(zp[:], zac[:], one_sb[:], start=True, stop=True)
            zsb = rt.tile([1, 1], F32)
            nc.scalar.copy(zsb[:], zp[:])
            nc.sync.dma_start(zs_h[:, :], zsb[:])

        # ---- top-2 + combine weights -----------------------------------
        mx = rt.tile([P, 32 * E], F32)
        ix = rt.tile([P, 32 * E], U32)
        for j in range(32):
            nc.vector.max_with_indices(mx[:, j * E:(j + 1) * E],
                                       ix[:, j * E:(j + 1) * E],
                                       lg[:, j * E:(j + 1) * E])
        mx3 = mx[:].rearrange("p (j e) -> p j e", e=E)
        ix3 = ix[:].rearrange("p (j e) -> p j e", e=E)
        i1f = rt.tile([P, 32], F32)
        nc.vector.tensor_copy(i1f[:], ix3[:, :, 0])
        i2f = rt.tile([P, 32], F32)
        nc.vector.tensor_copy(i2f[:], ix3[:, :, 1])
        d12 = rt.tile([P, 32], F32)
        nc.vector.tensor_tensor(d12[:], mx3[:, :, 0], mx3[:, :, 1], ALU.subtract)
        w1 = rt.tile([P, 32], F32)
        nc.scalar.activation(w1[:], d12[:], AF.Sigmoid)
        w2 = rt.tile([P, 32], F32)
        nc.vector.tensor_scalar(w2[:], w1[:], -1.0, 1.0, ALU.mult, ALU.add)
        f1 = rt.tile([P, 32], F32)
        nc.vector.tensor_scalar(f1[:], i1f[:], eid_sb[:], None, ALU.is_equal)
        f2 = rt.tile([P, 32], F32)
        nc.vector.tensor_scalar(f2[:], i2f[:], eid_sb[:], None, ALU.is_equal)
        fany = rt.tile([P, 32], F32)
        nc.vector.tensor_tensor(fany[:], f1[:], f2[:], ALU.add)
        wa = rt.tile([P, 32], F32)
        nc.vector.tensor_tensor(wa[:], f1[:], w1[:], ALU.mult)
        wb = rt.tile([P, 32], F32)
        nc.vector.tensor_tensor(wb[:], f2[:], w2[:], ALU.mult)
        ws = rt.tile([P, 32], F32)
        nc.vector.tensor_tensor(ws[:], wa[:], wb[:], ALU.add)
        fm1 = rt.tile([P, 32], F32)
        nc.vector.tensor_scalar(fm1[:], fany[:], -1.0, None, ALU.add)
        wmask = rt.tile([P, 32], F32)
        nc.vector.tensor_tensor(wmask[:], ws[:], fm1[:], ALU.add)
        idm = rt.tile([P, 32], F32)
        nc.vector.tensor_tensor(idm[:], fany[:], tok_sb[:], ALU.mult)
        imask = rt.tile([P, 32], F32)
        nc.vector.tensor_scalar(imask[:], idm[:], -1.0, None, ALU.add)

        # ---- 16-wrap repack: token t -> (t%16, t//16) ------------------
        ids16 = rt.tile([16, 256], F32)
        w16 = rt.tile([16, 256], F32)
        ids163 = ids16[:].rearrange("p (c a) -> p c a", a=8)
        w163 = w16[:].rearrange("p (c a) -> p c a", a=8)
        for a in range(8):
            nc.sync.dma_start(ids163[:, :, a], imask[16 * a:16 * (a + 1), :])
            nc.sync.dma_start(w163[:, :, a], wmask[16 * a:16 * (a + 1), :])

        # ---- compaction ------------------------------------------------
        cid16 = rt.tile([16, CW], F32)
        cnt_sb = rt.tile([1, 1], U32)
        nc.gpsimd.sparse_gather(cid16[:], ids16[:], num_found=cnt_sb[:])
        cw16 = rt.tile([16, CW], F32)
        cnt2_sb = rt.tile([1, 1], U32)
        nc.gpsimd.sparse_gather(cw16[:], w16[:], num_found=cnt2_sb[:])
        nc.sync.dma_start(cidx_h[:, :], cid16[:])
        nc.sync.dma_start(cnt_h[:, :], cnt_sb[:])

        # sanitize (-1 pad -> 0) and replicate indices for dma_gather
        cidm = rt.tile([16, CW], F32)
        nc.vector.tensor_scalar(cidm[:], cid16[:], 0.0, None, ALU.max)
        cid16i = rt.tile([16, CW], I16)
        nc.vector.tensor_copy(cid16i[:], cidm[:])
        cidi = rt.tile([P, CW], I16)
        for g in range(8):
            nc.sync.dma_start(cidi[16 * g:16 * (g + 1), :], cid16i[:])
        # combine weights -> slot-major [128, CPAD/128] layout
        cwP = rt.tile([P, CPAD // P], F32)
        cw3 = cw16[:].rearrange("p (k c) -> p k c", c=8)
        for c in range(8):
            nc.sync.dma_start(cwP[16 * c:16 * (c + 1), :], cw3[:, :, c])

        # ---- gather selected token rows (transposing gather) -----------
        xg = []
        for c, (off, ln) in enumerate(CHUNKS):
            t = xgp.tile([P, KH, ln], BF, tag=f"xg{c}", name=f"xg{c}")
            nc.gpsimd.dma_gather(
                t[:], xb_h[:, :], cidi[:, off // 16:(off + ln) // 16],
                num_idxs=ln, num_idxs_reg=ln, elem_size=H, transpose=True)
            xg.append(t)

        # ---- shared expert on this core's 512-token slice --------------
        with tc.tile_pool(name="ishp", bufs=1) as ishp:
            ish = ishp.tile([P, NI, TSH], BF)
            with tc.tile_pool(name="psG", bufs=2, space="PSUM") as psG:
                for i in range(NI):
                    gwc = wp.tile([P, H], BF, tag="w")
                    nc.sync.dma_start(gwc[:], sgw_h[i, :, :])
                    uwc = wp.tile([P, H], BF, tag="w")
                    nc.sync.dma_start(uwc[:], suw_h[i, :, :])
                    gps = psG.tile([P, TSH], F32, tag="g")
                    ups = psG.tile([P, TSH], F32, tag="u")
                    for k in range(KH):
                        nc.tensor.matmul(gps[:], gwc[:, k * P:(k + 1) * P],
                                         xs_sb[:, k, :],
                                         start=(k == 0), stop=(k == KH - 1))
                    for k in range(KH):
                        nc.tensor.matmul(ups[:], uwc[:, k * P:(k + 1) * P],
                                         xs_sb[:, k, :],
                                         start=(k == 0), stop=(k == KH - 1))
                    sg = actp.tile([P, TSH], F32, tag="sg")
                    nc.scalar.activation(sg[:], gps[:], AF.Sigmoid)
                    t1 = actp.tile([P, TSH], F32, tag="t1")
                    nc.vector.tensor_tensor(t1[:], sg[:], gps[:], ALU.mult)
                    nc.vector.tensor_tensor(ish[:, i, :], t1[:], ups[:], ALU.mult)
            with tc.tile_pool(name="psDs", bufs=1, space="PSUM") as psD:
                dps = [psD.tile([P, 512], F32, tag=f"d{q}", name=f"dps{q}") for q in range(8)]
                for i in range(NI):
                    dwc = wp.tile([P, H], BF, tag="w")
                    nc.sync.dma_start(dwc[:], sdw_h[i, :, :])
                    for cs in range(4):
                        lhs = ish[:, i, cs * P:(cs + 1) * P]
                        for hh in range(2):
                            nc.tensor.matmul(dps[2 * cs + hh][:], lhs,
                                             dwc[:, hh * 512:(hh + 1) * 512],
                                             start=(i == 0), stop=(i == NI - 1))
                for cs in range(4):
                    ysb = outp.tile([P, H], F32, tag="ysh")
                    for hh in range(2):
                        nc.scalar.copy(ysb[:, hh * 512:(hh + 1) * 512],
                                       dps[2 * cs + hh][:])
                    nc.sync.dma_start(ysh_h[cs * P:(cs + 1) * P, :], ysb[:])

        # ---- routed expert: gate/up on compact tokens ------------------
        with tc.tile_pool(name="innp", bufs=1) as innp:
            inner = innp.tile([P, NI * CPAD], BF)
            innv = inner[:].rearrange("p (i c) -> p i c", c=CPAD)
            with tc.tile_pool(name="psGU", bufs=1, space="PSUM") as psGU:
                for i in range(NI):
                    gwc = wp.tile([P, H], BF, tag="w")
                    nc.sync.dma_start(gwc[:], gw_h[i, :, :])
                    uwc = wp.tile([P, H], BF, tag="w")
                    nc.sync.dma_start(uwc[:], uw_h[i, :, :])
                    gp = [psGU.tile([P, CHUNKS[c][1]], F32, tag=f"g{c}", name=f"gp{c}") for c in range(NCH)]
                    up = [psGU.tile([P, CHUNKS[c][1]], F32, tag=f"u{c}", name=f"up{c}") for c in range(NCH)]
                    for k in range(KH):
                        for c in range(NCH):
                            nc.tensor.matmul(gp[c][:], gwc[:, k * P:(k + 1) * P],
                                             xg[c][:, k, :],
                                             start=(k == 0), stop=(k == KH - 1))
                    for k in range(KH):
                        for c in range(NCH):
                            nc.tensor.matmul(up[c][:], uwc[:, k * P:(k + 1) * P],
                                             xg[c][:, k, :],
                                             start=(k == 0), stop=(k == KH - 1))
                    for c, (off, ln) in enumerate(CHUNKS):
                        sg = actp.tile([P, 512], F32, tag="sg")
                        nc.scalar.activation(sg[:, :ln], gp[c][:], AF.Sigmoid)
                        t1 = actp.tile([P, 512], F32, tag="t1")
                        nc.vector.tensor_tensor(t1[:, :ln], sg[:, :ln], gp[c][:], ALU.mult)
                        nc.vector.tensor_tensor(innv[:, i, off:off + ln],
                                                t1[:, :ln], up[c][:], ALU.mult)

            # ---- routed down + combine-weight scaling ------------------
            with tc.tile_pool(name="psDr", bufs=1, space="PSUM") as psD2:
                for cs0, ncs in CSGRP:
                    dps = [psD2.tile([P, 512], F32, tag=f"d{q}", name=f"dpr{q}")
                           for q in range(2 * ncs)]
                    for i in range(NI):
                        dwc = wp.tile([P, H], BF, tag="w")
                        nc.sync.dma_start(dwc[:], dw_h[i, :, :])
                        for lcs in range(ncs):
                            cs = cs0 + lcs
                            lhs = innv[:, i, cs * P:(cs + 1) * P]
                            for hh in range(2):
                                nc.tensor.matmul(dps[2 * lcs + hh][:], lhs,
                                                 dwc[:, hh * 512:(hh + 1) * 512],
                                                 start=(i == 0), stop=(i == NI - 1))
                    for lcs in range(ncs):
                        cs = cs0 + lcs
                        ysb = outp.tile([P, H], F32, tag="y")
                        for hh in range(2):
                            nc.vector.tensor_scalar(
                                ysb[:, hh * 512:(hh + 1) * 512],
                                dps[2 * lcs + hh][:], cwP[:, cs:cs + 1], None,
                                ALU.mult)
                        nc.sync.dma_start(y_h[cs * P:(cs + 1) * P, :], ysb[:])

    nc.compile()
    return nc


def get_nc():
    if "nc" not in _NC_CACHE:
        _NC_CACHE["nc"] = _build_nc()
    return _NC_CACHE["nc"]


def _prep_gu(w):
    # [I, H] -> [NI, P, H]; [it, p, k*128+i] = w[128*it+i, 128*k+p]
    return np.ascontiguousarray(
        w.reshape(NI, P, KH, P).transpose(0, 3, 2, 1).reshape(NI, P, H)
    ).astype(BF16)


def _prep_d(w):
    # [H, I] -> [NI, P, H]; [ic, p, h] = w[h, 128*ic+p]
    return np.ascontiguousarray(w.T.reshape(NI, P, H)).astype(BF16)


def prepare_in_maps(hidden_states, router_w, gate_w, up_w, down_w,
                    sh_gate_w, sh_up_w, sh_down_w):
    x = np.asarray(hidden_states, np.float32).reshape(T, H)
    xT = np.ascontiguousarray(x.T)
    xr = np.ascontiguousarray(xT.reshape(KH, P, T).transpose(1, 0, 2))
    xb = x.astype(BF16)
    rw = np.ascontiguousarray(
        np.asarray(router_w, np.float32).T.reshape(KH, P, E).transpose(1, 0, 2))
    sgw = _prep_gu(np.asarray(sh_gate_w, np.float32))
    suw = _prep_gu(np.asarray(sh_up_w, np.float32))
    sdw = _prep_d(np.asarray(sh_down_w, np.float32))
    tok1 = (np.arange(32, dtype=np.float32)[None, :] * P
            + np.arange(P, dtype=np.float32)[:, None] + 1.0)
    ones = np.ones((P, 1), np.float32)
    in_maps = []
    for e in range(E):
        xs = np.ascontiguousarray(
            xT[:, e * TSH:(e + 1) * TSH].reshape(KH, P, TSH).transpose(1, 0, 2)
        ).astype(BF16)
        in_maps.append(dict(
            xr=xr, xb=xb, xs=xs, rwt=rw,
            gw=_prep_gu(np.asarray(gate_w[e], np.float32)),
            uw=_prep_gu(np.asarray(up_w[e], np.float32)),
            dw=_prep_d(np.asarray(down_w[e], np.float32)),
            sgw=sgw, suw=suw, sdw=sdw,
            eid=np.full((P, 1), float(e), np.float32),
            tok1=tok1, ones=ones,
        ))
    return in_maps


def combine_outputs(outs):
    final = np.zeros((T, H), np.float32)
    counts = np.zeros(E, np.float32)
    for e in range(E):
        o = outs[e]
        final[e * TSH:(e + 1) * TSH] = np.asarray(o["ysh"], np.float32)
    for e in range(E):
        o = outs[e]
        cnt = int(np.asarray(o["cnt"]).ravel()[0])
        counts[e] = cnt
        ids = np.asarray(o["cidx"]).reshape(16, CW).T.ravel()[:cnt].astype(np.int64)
        final[ids] += np.asarray(o["y"], np.float32)[:cnt]
    loads = np.concatenate([counts, [float(T)]]).astype(np.float32)
    loads_n = loads / loads.sum()
    lb = np.mean((loads_n - 1.0 / (E + 1)) ** 2)
    z = float(np.asarray(outs[0]["zs"]).ravel()[0]) / float(T)
    aux = np.float32(0.01 * lb + 0.01 * z)
    return final.reshape(2, 2048, H), aux


def kernel(hidden_states, router_w, gate_w, up_w, down_w,
           sh_gate_w, sh_up_w, sh_down_w):
    in_maps = prepare_in_maps(hidden_states, router_w, gate_w, up_w, down_w,
                              sh_gate_w, sh_up_w, sh_down_w)
    nc = get_nc()
    res = run_bass_kernel_spmd(nc, in_maps, list(range(E)),
                               trace=bool(os.environ.get("MOE_TRACE")))
    if os.environ.get("MOE_TRACE"):
        _NC_CACHE["last_results"] = res
    final, aux = combine_outputs(res.results)
    return final, aux
